# revision 43
# baseline (speedup 1.0000x reference)
"""Multi-head attention (B=2, S=2048, E=1024, H=16) on 8 Trainium2 cores.

Sharding: core c -> (batch b = c//4, head-group g = c%4 of 4 heads).
Each core computes Q/K/V projections for its 4 heads (256 features),
full attention for those heads, and a partial output projection
(256 rows of Wo). Host sums the 4 partials per batch element and adds bo.

The schedule is built around the Scalar(ACT) engine's exp throughput,
which is the hard floor (~142us for 16.7M score elements per core):

- x tensors arrive as token-chunks, each a separate SBUF tile fed by one
  contiguous 2D DMA (chunk-major DRAM layout prepared on the host), so
  the tile dependency tracker sees exact per-chunk producer/consumer
  edges and the first scores matmul can issue as soon as the first
  chunks land. Chunks are spread across the three DGE queues (sync-HW,
  scalar-HW, gpsimd-SW) in first-use-deadline order.
- K/V projections are decomposed into per-token-tile units and Q into
  per-query-block units; projection work is interleaved into the
  attention loop as fillers between the scores matmul and the P@V
  matmuls, filling PE slack while the ACT engine streams exps.
- y-projection blocks are likewise spread as fillers during the
  following query-block instead of running as a lump at each block end.

On-chip layouts (per core):
  qt/kt: (128 feat-part, pair, 2048 tok)  transposed proj outputs; the
         128 partitions hold two heads (64+64) per pair index.
  v:     (128 tok-part, 16 tok-tiles, 4*65): per head 64 dims plus a
         "ones" column produced by an augmented V projection (extra
         output feature with zero weights and bias 1.0); P @ V_aug then
         also yields the softmax denominator row for free.
  scores are computed transposed (key-pos on partitions, query on free)
  so exp runs on ACT along the free dim and P tiles feed P@V directly as
  the moving operand; the two heads of a pair run as PE row-groups.
"""

import numpy as np

B, S, E, H = 2, 2048, 1024, 16
D = 64
NCORES = 8
FPC = 256  # features (head dims) per core = 4 heads
VW = 4 * 65  # V-projection output width incl. ones columns
NW = 512  # attention query-block width (psum bank = 512 fp32)

_PROGRAMS = {}
LAST_RESULT = None
TRACE = False
TRACE_DIR = None


def _build():
    import concourse.tile as tile
    from concourse import bacc, mybir

    f32 = mybir.dt.float32
    DT = mybir.dt.bfloat16
    Exp = mybir.ActivationFunctionType.Exp
    Copy = mybir.ActivationFunctionType.Copy

    nc = bacc.Bacc("TRN2", target_bir_lowering=False, debug=False,
                   num_devices=NCORES)

    xq_ap = nc.dram_tensor("xq", [128, 8, 8, 256], DT,
                           kind="ExternalInput").ap()
    xk_ap = nc.dram_tensor("xk", [128, 8, 8, 256], DT,
                           kind="ExternalInput").ap()
    xv_ap = nc.dram_tensor("xv", [128, 8, 8, 256], DT,
                           kind="ExternalInput").ap()
    wq_ap = nc.dram_tensor("wq", [128, 2, 8, 128], DT,
                           kind="ExternalInput").ap()
    wk_ap = nc.dram_tensor("wk", [128, 2, 8, 128], DT,
                           kind="ExternalInput").ap()
    wv_ap = nc.dram_tensor("wv", [128, 8, VW], DT, kind="ExternalInput").ap()
    wo_ap = nc.dram_tensor("wo", [128, 2, E], DT, kind="ExternalInput").ap()
    bqk_ap = nc.dram_tensor("bqk", [128, 4], f32, kind="ExternalInput").ap()
    bv_ap = nc.dram_tensor("bv", [1, VW], DT, kind="ExternalInput").ap()
    ones_ap = nc.dram_tensor("ones", [1, 128], DT, kind="ExternalInput").ap()
    y_ap = nc.dram_tensor("y", [S, E], f32, kind="ExternalOutput").ap()

    with tile.TileContext(nc) as tc:
        with tc.tile_pool(name="persist", bufs=1) as persist, \
             tc.tile_pool(name="pt", bufs=20) as ptpool, \
             tc.tile_pool(name="sm", bufs=2) as smpool, \
             tc.tile_pool(name="ysb", bufs=2) as ypool, \
             tc.tile_pool(name="projps", bufs=2, space="PSUM") as projps, \
             tc.tile_pool(name="scps", bufs=2, space="PSUM") as scps, \
             tc.tile_pool(name="pvps", bufs=2, space="PSUM") as pvps:
            wq_p = [persist.tile([128, 8, 128], DT, name=f"wq_{p}")
                    for p in range(2)]
            wk_p = [persist.tile([128, 8, 128], DT, name=f"wk_{p}")
                    for p in range(2)]
            wv_sb = persist.tile([128, 8, VW], DT, name="wv_sb")
            wo_sb = persist.tile([128, 2, E], DT, name="wo_sb")
            bqk_sb = persist.tile([128, 4], f32, name="bqk_sb")
            bv_sb = persist.tile([1, VW], DT, name="bv_sb")
            ones_sb = persist.tile([1, 128], DT, name="ones_sb")
            qt_sb = persist.tile([128, 2, S], DT, name="qt_sb")
            kt_sb = persist.tile([128, 2, S], DT, name="kt_sb")
            v_sb = persist.tile([128, 16, VW], DT, name="v_sb")
            at_sb = persist.tile([128, 2, S], DT, name="at_sb")
            xq_c = [persist.tile([128, 8, 256], DT, name=f"xq_c{c}")
                    for c in range(8)]
            xk_c = [persist.tile([128, 8, 256], DT, name=f"xk_c{c}")
                    for c in range(8)]
            xv_c = [persist.tile([128, 8, 256], DT, name=f"xv_c{c}")
                    for c in range(8)]
            scr = persist.tile([128, NW], DT, name="scr")
            ones32 = ones_sb[:, 0:64]

            # ---- DMA schedule ----
            # The scalar(ACT) engine must stay nearly DMA-free: queued
            # dma_starts block the exp stream behind them on the in-order
            # sequencer once the DGE queue backs up. It gets only the three
            # transfers needed in the first ~15us; everything else is split
            # between sync-HW and gpsimd-SW in first-use-deadline order.
            nc.scalar.dma_start(xq_c[1], xq_ap[:, 1])
            nc.scalar.dma_start(xv_c[0], xv_ap[:, 0])
            nc.scalar.dma_start(xv_c[1], xv_ap[:, 1])
            # gpsimd (SW queue)
            nc.gpsimd.dma_start(bqk_sb, bqk_ap)
            nc.gpsimd.dma_start(bv_sb, bv_ap)
            nc.gpsimd.dma_start(ones_sb, ones_ap)
            nc.gpsimd.dma_start(wq_p[0], wq_ap[:, 0])
            nc.gpsimd.dma_start(wk_p[0], wk_ap[:, 0])
            nc.gpsimd.dma_start(xk_c[1], xk_ap[:, 1])
            nc.gpsimd.dma_start(wv_sb, wv_ap)
            for t in ("v2", "k3", "k4", "v5", "v6", "v7"):
                src = xk_ap if t[0] == "k" else xv_ap
                dst = xk_c if t[0] == "k" else xv_c
                c = int(t[1])
                nc.gpsimd.dma_start(dst[c], src[:, c])
            nc.gpsimd.dma_start(xq_c[2], xq_ap[:, 2])
            nc.gpsimd.dma_start(wq_p[1], wq_ap[:, 1])
            nc.gpsimd.dma_start(wk_p[1], wk_ap[:, 1])
            nc.gpsimd.dma_start(xq_c[4], xq_ap[:, 4])
            nc.gpsimd.dma_start(xq_c[6], xq_ap[:, 6])
            nc.gpsimd.dma_start(wo_sb, wo_ap)
            # sync (HW queue)
            nc.sync.dma_start(xq_c[0], xq_ap[:, 0])
            for t in ("k0", "k2", "v3", "v4", "k5", "k6", "k7"):
                src = xk_ap if t[0] == "k" else xv_ap
                dst = xk_c if t[0] == "k" else xv_c
                c = int(t[1])
                nc.sync.dma_start(dst[c], src[:, c])
            nc.sync.dma_start(xq_c[3], xq_ap[:, 3])
            nc.sync.dma_start(xq_c[5], xq_ap[:, 5])
            nc.sync.dma_start(xq_c[7], xq_ap[:, 7])

            # ---- projection emission units (fillers) ----
            # all projection units share one [128, 512] psum tag (1 bank)
            def emit_qproj(p, qb):
                # Q proj for pair p, 512-token query block qb (2 x chunks)
                pj = projps.tile([128, NW], f32, tag="pj",
                                 name=f"q_{p}_{qb}")
                for h in range(2):
                    for kt in range(8):
                        nc.tensor.matmul(pj[:, h * 256:(h + 1) * 256],
                                         wq_p[p][:, kt, :],
                                         xq_c[2 * qb + h][:, kt, :],
                                         start=(kt == 0), stop=(kt == 7))
                nc.vector.tensor_scalar_add(
                    qt_sb[:, p, qb * NW:(qb + 1) * NW], pj,
                    bqk_sb[:, p:p + 1])

            def emit_kproj(p, c):
                # K proj for pair p, 256-token chunk c
                pj = projps.tile([128, NW], f32, tag="pj",
                                 name=f"k_{p}_{c}")[:, 0:256]
                for kt in range(8):
                    nc.tensor.matmul(pj, wk_p[p][:, kt, :],
                                     xk_c[c][:, kt, :],
                                     start=(kt == 0), stop=(kt == 7))
                nc.vector.tensor_scalar_add(
                    kt_sb[:, p, c * 256:(c + 1) * 256], pj,
                    bqk_sb[:, 2 + p:3 + p])

            def emit_vproj(t):
                # V proj for 128-token tile t, all 4 heads + ones columns
                pj = projps.tile([128, NW], f32, tag="pj",
                                 name=f"v_{t}")[:, 0:VW]
                nc.tensor.matmul(pj, ones_sb, bv_sb, start=True, stop=False)
                o = (t % 2) * 128
                for kt in range(8):
                    nc.tensor.matmul(pj,
                                     xv_c[t // 2][:, kt, o:o + 128],
                                     wv_sb[:, kt, :],
                                     start=False, stop=(kt == 7))
                nc.vector.tensor_copy(v_sb[:, t, :], pj)

            def emit_yproj_tile(mt, queues, ceng=None):
                # output proj for 128-token tile mt, all 1024 out features;
                # each 512-wide half goes out on its own queue
                yo = ypool.tile([128, E], f32, tag="yo", name=f"yo_{mt}")
                for nb in range(2):
                    yp = projps.tile([128, NW], f32, tag="pj",
                                     name=f"yp_{mt}_{nb}")
                    for p2 in range(2):
                        nc.tensor.matmul(
                            yp, at_sb[:, p2, mt * 128:(mt + 1) * 128],
                            wo_sb[:, p2, nb * NW:(nb + 1) * NW],
                            start=(p2 == 0), stop=(p2 == 1))
                    if nb and ceng == "act":
                        # ACT engine is free once the exp stream has ended
                        nc.scalar.activation(yo[:, nb * NW:(nb + 1) * NW],
                                             yp, Copy)
                    else:
                        nc.vector.tensor_copy(yo[:, nb * NW:(nb + 1) * NW],
                                              yp)
                    queues[nb].dma_start(
                        y_ap[mt * 128:(mt + 1) * 128, nb * NW:(nb + 1) * NW],
                        yo[:, nb * NW:(nb + 1) * NW])

            def emit_denom_a(p, qb, pvt, act=False):
                # denominator rows -> small SBUF tiles (DVE, or ACT when the
                # exp stream is over)
                out = []
                for hh in range(2):
                    denr = smpool.tile([1, NW], DT, tag="denr",
                                       name=f"dn_{qb}_{p}_{hh}")
                    if act and hh:
                        nc.scalar.activation(denr, pvt[hh][64:65, :], Copy)
                    else:
                        nc.vector.tensor_copy(denr, pvt[hh][64:65, :])
                    out.append(denr)
                return out

            def emit_denom_b(p, qb, pvt, denrs):
                # broadcast to 64 partitions (PE) -> reciprocal -> normalize
                # numerators into at_sb
                qsl = slice(qb * NW, (qb + 1) * NW)
                for hh in range(2):
                    rb = projps.tile([128, NW], f32, tag="pj",
                                     name=f"rb_{qb}_{p}_{hh}")[0:64, :]
                    nc.tensor.matmul(rb, ones32, denrs[hh],
                                     start=True, stop=True)
                    rbs = smpool.tile([64, NW], f32, tag="rbs",
                                      name=f"rbs_{qb}_{p}_{hh}")
                    nc.vector.reciprocal_approx_fast(rbs, rb)
                    nc.vector.tensor_mul(at_sb[64 * hh:64 * hh + 64, p, qsl],
                                         pvt[hh][0:64, :], rbs)

            # ---- P@V cascade state ----
            # block b's P@V matmuls run one block later (lag 16) on retained
            # exp tiles, so every block's PE load stays level and the first
            # block needs only xq/xk while the DMA stream catches up.
            ptts = {}
            pvt_of = {}
            denr_of = {}

            def emit_pv(b, kt):
                p = b // 4
                if b not in pvt_of:
                    pvt_of[b] = [pvps.tile([65, NW], f32, tag="pv",
                                           name=f"pv_{b}_{hh}")
                                 for hh in range(2)]
                for hh in range(2):
                    h = 2 * p + hh
                    nc.tensor.matmul(pvt_of[b][hh],
                                     v_sb[:, kt, 65 * h:65 * h + 65],
                                     ptts[(b, kt)][:, NW * hh:NW * hh + NW],
                                     start=(kt == 0), stop=(kt == 15))

            def emit_da(b, act=False):
                denr_of[b] = emit_denom_a(b // 4, b % 4, pvt_of[b], act)

            def emit_db(b):
                emit_denom_b(b // 4, b % 4, pvt_of[b], denr_of[b])

            # sched[b][iter] -> closures emitted after that iteration's
            # scores+exp
            sched = {b: {kt: [] for kt in range(16)} for b in range(8)}

            def add(b, it, fn):
                sched[b][it].append(fn)

            def add_pvs(b, src):
                # 16 P@V units of block `src` over iters 2..15 of block b
                its = [2, 2, 3, 3] + list(range(4, 16))
                for kt in range(16):
                    add(b, its[kt], lambda s=src, k=kt: emit_pv(s, k))
                add(b, 15, lambda s=src: emit_da(s))

            yqs = [[nc.sync, nc.gpsimd], [nc.gpsimd, nc.sync]]
            # B0: K p0 chunks + V tiles 0..7 as xv lands + Q qb1
            for c in range(1, 8):
                add(0, 2 * c - 1, lambda cc=c: emit_kproj(0, cc))
            for t in range(8):
                add(0, 3 + t, lambda tt=t: emit_vproj(tt))
            add(0, 12, lambda: emit_qproj(0, 1))
            # B1: V 8..15, PV(b0), Q qb2
            for t in range(8, 16):
                add(1, t - 8, lambda tt=t: emit_vproj(tt))
            add_pvs(1, 0)
            add(1, 8, lambda: emit_qproj(0, 2))
            # B2: denom(b0), PV(b1), K p1 0..3, Q qb3
            add(2, 0, lambda: emit_db(0))
            add_pvs(2, 1)
            for c in range(4):
                add(2, 5 + 2 * c, lambda cc=c: emit_kproj(1, cc))
            add(2, 13, lambda: emit_qproj(0, 3))
            # B3: denom(b1), PV(b2), K p1 4..7, Q p1 qb0
            add(3, 0, lambda: emit_db(1))
            add_pvs(3, 2)
            for c in range(4, 8):
                add(3, 5 + 2 * (c - 4), lambda cc=c: emit_kproj(1, cc))
            add(3, 13, lambda: emit_qproj(1, 0))
            # B4, B5: denom, PV, Q
            add(4, 0, lambda: emit_db(2))
            add_pvs(4, 3)
            add(4, 7, lambda: emit_qproj(1, 1))
            add(5, 0, lambda: emit_db(3))
            add_pvs(5, 4)
            add(5, 7, lambda: emit_qproj(1, 2))
            # B6: denom(b4), PV(b5), Q p1 qb3, yproj(qb0)
            add(6, 0, lambda: emit_db(4))
            add_pvs(6, 5)
            add(6, 5, lambda: emit_qproj(1, 3))
            for i, mt in enumerate(range(0, 4)):
                add(6, 7 + 2 * i,
                    lambda m=mt, q=yqs[i % 2]: emit_yproj_tile(m, q))
            # B7: denom(b5), PV(b6) front-loaded, denom(b6), yproj(qb1),
            # first P@V pair of b7
            add(7, 0, lambda: emit_db(5))
            its7 = [2, 2, 3, 3, 4, 4, 5, 5, 6, 6, 7, 7, 8, 8, 9, 9]
            for kt in range(16):
                add(7, its7[kt], lambda k=kt: emit_pv(6, k))
            add(7, 9, lambda: emit_da(6))
            add(7, 10, lambda: emit_db(6))
            for i, mt in enumerate(range(4, 8)):
                add(7, 11 + i,
                    lambda m=mt, q=yqs[i % 2]: emit_yproj_tile(m, q))
            add(7, 15, lambda: emit_pv(7, 0))
            add(7, 15, lambda: emit_pv(7, 1))

            # ---- main attention loop ----
            # virtual timeline (ms) pinning the static scheduler's order
            def slot(g):
                if g < 16:
                    return 0.016 + 0.00135 * g
                return 0.0376 + 0.00115 * (g - 16)

            # warm-up: keep the PE clocked up while the first x chunks land
            with tc.tile_wait_until(0.002):
                nc.vector.memset(scr, 0.0)
                for i in range(24):
                    dm = projps.tile([128, NW], f32, tag="pj",
                                     name=f"warm_{i}")
                    nc.tensor.matmul(dm, scr[:, 0:128], scr,
                                     start=True, stop=True)
            with tc.tile_wait_until(0.0115):
                emit_qproj(0, 0)
            with tc.tile_wait_until(0.013):
                emit_kproj(0, 0)

            for b in range(8):
                p, qb = b // 4, b % 4
                qsl = slice(qb * NW, (qb + 1) * NW)
                for kt in range(16):
                    with tc.tile_wait_until(slot(b * 16 + kt)):
                        s_ = scps.tile([128, 2 * NW], f32, tag="sc",
                                       name=f"sc_{b}_{kt}")
                        for hh in range(2):
                            nc.tensor.matmul(
                                s_[:, NW * hh:NW * hh + NW],
                                kt_sb[64 * hh:64 * hh + 64, p,
                                      kt * 128:(kt + 1) * 128],
                                qt_sb[64 * hh:64 * hh + 64, p, qsl],
                                start=True, stop=True)
                        ptt = ptpool.tile([128, 2 * NW], DT, tag="pt",
                                          name=f"pt_{b}_{kt}")
                        nc.scalar.activation(ptt, s_, Exp, scale=0.125)
                        ptts[(b, kt)] = ptt
                        for fn in sched[b][kt]:
                            fn()
            with tc.tile_wait_until(slot(127) + 0.0006):
                # tail: remaining PV of the final block, its denom chain
                # interleaved with yproj(qb2) so the PE never idles, then
                # the final yproj(qb3). Scalar's HW queue and the ACT
                # engine are free once the exps end.
                tq = [[nc.sync, nc.scalar], [nc.gpsimd, nc.scalar],
                      [nc.sync, nc.scalar], [nc.gpsimd, nc.scalar]]
                for kt in range(2, 16):
                    emit_pv(7, kt)
                emit_da(7, act=True)
                emit_yproj_tile(8, tq[0], ceng="act")
                emit_db(7)
                for i, mt in enumerate(range(9, 12)):
                    emit_yproj_tile(mt, tq[1 + i], ceng="act")
                for i, mt in enumerate(range(12, 16)):
                    emit_yproj_tile(mt, tq[i], ceng="act")

    nc.compile()
    return nc


def _get_program():
    if "k" not in _PROGRAMS:
        _PROGRAMS["k"] = _build()
    return _PROGRAMS["k"]


def kernel(q, k, v, mask, Wq, bq, Wk, bk, Wv, bv, Wo, bo):
    global LAST_RESULT
    from concourse.bass_utils import run_bass_kernel_spmd
    import ml_dtypes

    nc = _get_program()
    cdt = ml_dtypes.bfloat16

    def prep(a):
        return np.ascontiguousarray(np.asarray(a).astype(cdt))

    q = np.asarray(q); k = np.asarray(k); v = np.asarray(v)
    Wq = np.asarray(Wq); Wk = np.asarray(Wk); Wv = np.asarray(Wv)
    Wo = np.asarray(Wo)
    bq = np.asarray(bq); bk = np.asarray(bk); bv = np.asarray(bv)
    bo = np.asarray(bo)

    def x_layout(a, b, nchunk):
        # [128 part, chunk, kt, tok]: part p, kt holds feature row kt*128+p
        # of x^T; token chunks are contiguous per partition for 2D DMA
        W = S // nchunk
        A = a[b].T.reshape(8, 128, nchunk, W)  # (kt, part, chunk, tok)
        return prep(A.transpose(1, 2, 0, 3))

    def wqk_layout(W, r0):
        # [part, pair, kt, m]: W.T[kt*128+part, pair*128+m]
        A = W[r0:r0 + FPC, :].T.reshape(8, 128, 2, 128)
        return prep(A.transpose(1, 2, 0, 3))

    in_maps = []
    for core in range(NCORES):
        b, g = core // 4, core % 4
        r0 = g * FPC

        WvT = Wv[r0:r0 + FPC, :].T  # (E, 256)
        Wv_aug = np.zeros((E, VW), np.float32)
        bv_aug = np.zeros((1, VW), np.float32)
        for h in range(4):
            Wv_aug[:, 65 * h:65 * h + 64] = WvT[:, 64 * h:64 * h + 64]
            bv_aug[0, 65 * h:65 * h + 64] = bv[r0 + 64 * h:r0 + 64 * h + 64]
            bv_aug[0, 65 * h + 64] = 1.0
        Wo_l = Wo[:, r0:r0 + FPC].T.reshape(2, 128, E).transpose(1, 0, 2)

        in_maps.append({
            "xq": x_layout(q, b, 8),
            "xk": x_layout(k, b, 8),
            "xv": x_layout(v, b, 8),
            "wq": wqk_layout(Wq, r0),
            "wk": wqk_layout(Wk, r0),
            "wv": prep(Wv_aug.reshape(8, 128, VW).transpose(1, 0, 2)),
            "wo": prep(Wo_l),
            "bqk": np.stack([bq[r0:r0 + 128], bq[r0 + 128:r0 + FPC],
                             bk[r0:r0 + 128], bk[r0 + 128:r0 + FPC]],
                            axis=1).astype(np.float32),
            "bv": prep(bv_aug),
            "ones": np.ones((1, 128), cdt),
        })

    kwargs = {}
    if TRACE:
        kwargs = {"trace": True, "tmpdir": TRACE_DIR}
    res = run_bass_kernel_spmd(nc, in_maps, list(range(NCORES)), **kwargs)
    LAST_RESULT = res

    y = np.zeros((B, S, E), np.float32)
    for core in range(NCORES):
        y[core // 4] += res.results[core]["y"]
    y += bo.astype(np.float32)
    return y
